# revision 8
# baseline (speedup 1.0000x reference)
"""CSWin Transformer block kernel for 8 Trainium2 NeuronCores (v2).

Data-parallel over batch: 32 images -> 4 per core. Each core runs the full
block (LN1, qkv, cross-shaped window attention with LePE, proj, residual,
LN2, MLP, residual) on its shard, pipelined across images.

Layouts per core (L = 3136 tokens per image):
  - token-major: (112 tokens on partitions, 128 ch free), 28 tiles per image.
  - channel-major mixed tensors (qT/kT/vT/tmp_att): rows 0:64 hold branch 0
    in w-major column order (col = 56*w + h), rows 64:128 hold branch 1 in
    h-major order (col = 56*h + w). Window w of either branch is columns
    [112*w, 112*w+112). Branch-0 w-major columns are produced directly by
    the qkv matmuls using strided rhs access patterns on the single h-major
    LN1 tensor (no reorder copies).
  - LePE runs as 9 aligned bf16 STT taps over stripe views (s, y, x) using
    two DMA-produced x-shifted copies of v; the centre tap doubles as the
    w->h reorder for branch 0 (strided output AP into att_h).

Engine budget: exp/gelu/LN-applies/q/v/proj evacs on Scalar; LePE, resid,
softmax-reciprocal+normalise, k/f2/transpose evacs on Vector; LN1/LN2
sum+sumsq on GpSimd (accumulate-with-dump); matmuls/transposes on Tensor.
LN gammas folded into the following matmul weights host-side; k-bias dropped
(softmax-invariant); LePE conv bias + v-bias folded into the proj bias.
"""
import sys
sys.path.insert(0, "/opt/trn_rl_repo")
import os
import numpy as np
import concourse.bass as bass
from concourse import bacc
import concourse.tile as tile
from concourse import mybir
from concourse.bass_utils import run_bass_kernel_spmd
from concourse.masks import make_identity

F32 = mybir.dt.float32
BF16 = mybir.dt.bfloat16
AL = mybir.AluOpType
AF = mybir.ActivationFunctionType

N_CORES = 8
B, RESO, C = 32, 56, 128
L = RESO * RESO            # 3136
IMG = B // N_CORES         # 4 images per core
T = IMG * L                # 12544 tokens per core
PT = 112                   # tokens per token-major tile
NTI = L // PT              # 28 token tiles per image
CK = 448                   # tokens per C-major chunk
NCK = L // CK              # 7 chunks per image
NWIN = 28                  # windows per image per branch
WT = 112                   # tokens per window
HD = 32
EPS = 1e-5
RCP_C = 1.0 / C


def build(nc, dbg=()):
    x_in = nc.declare_dram_parameter("x", [T, C], F32, isOutput=False)
    wqkv_in = nc.declare_dram_parameter("wqkv", [C, 3 * C], F32, isOutput=False)
    wproj_in = nc.declare_dram_parameter("wproj", [C, C], F32, isOutput=False)
    wfc1_in = nc.declare_dram_parameter("wfc1", [C, 4 * C], F32, isOutput=False)
    wfc2_in = nc.declare_dram_parameter("wfc2", [4 * C, C], F32, isOutput=False)
    # vecs cols: 0:s2q 1:s2k(unused) 2:s2v 3:projb 4:fc2b 5:eps 6..14:taps 15..18:fc1b
    vecs_in = nc.declare_dram_parameter("vecs", [C, 19], F32, isOutput=False)
    out_t = nc.declare_dram_parameter("out", [T, C], F32, isOutput=True)
    dbg_outs = {}
    for name, shape in dbg:
        dbg_outs[name] = nc.declare_dram_parameter(name, shape, F32, isOutput=True)

    tc = tile.TileContext(nc)
    with tc:
        with (
            tc.tile_pool(name="consts", bufs=1) as consts,
            tc.tile_pool(name="glob", bufs=1) as glob,
            tc.tile_pool(name="pimg", bufs=2) as pimg,
            tc.tile_pool(name="pim1", bufs=1) as pim1,
            tc.tile_pool(name="small", bufs=2) as small,
            tc.tile_pool(name="psU", bufs=3, space="PSUM") as psU,
            tc.tile_pool(name="psT", bufs=2, space="PSUM") as psT,
        ):
            _body(nc, consts, glob, pimg, pim1, small, psU, psT,
                  x_in, wqkv_in, wproj_in, wfc1_in, wfc2_in, vecs_in,
                  out_t, dbg_outs)
    return nc


def _body(nc, consts, glob, pimg, pim1, small, psU, psT,
          x_in, wqkv_in, wproj_in, wfc1_in, wfc2_in, vecs_in, out_t, dbg_outs):
    # ---------------- constants / weights ----------------
    identb = consts.tile([128, 128], BF16)
    make_identity(nc, identb[:])
    onesb = consts.tile([WT, 32], BF16)
    nc.vector.memset(onesb[:], 1.0)
    wqkv = consts.tile([C, 3 * C], BF16)
    nc.gpsimd.dma_start(out=wqkv[:], in_=wqkv_in[:])
    wproj = consts.tile([C, C], BF16)
    nc.gpsimd.dma_start(out=wproj[:], in_=wproj_in[:])
    wfc1 = consts.tile([C, 4 * C], BF16)
    nc.gpsimd.dma_start(out=wfc1[:], in_=wfc1_in[:])
    wfc2 = consts.tile([C, 4, C], BF16)
    nc.gpsimd.dma_start(out=wfc2[:], in_=wfc2_in.rearrange("(k p) o -> p k o", p=C))
    vecs = consts.tile([C, 19], F32)
    nc.sync.dma_start(out=vecs[:], in_=vecs_in[:])
    s2q = vecs[:, 0:1]
    s2v = vecs[:, 2:3]
    projb, fc2b, epsv = vecs[:, 3:4], vecs[:, 4:5], vecs[:, 5:6]
    taps = [vecs[:, 6 + i:7 + i] for i in range(9)]
    fc1b = [vecs[:, 15 + h:16 + h] for h in range(4)]

    def ln_stats(xsrc, mvs):
        """Vector bn_stats/bn_aggr: per-tile mean/var into mvs [PT, NTI, 2]."""
        for ti in range(NTI):
            st = small.tile([PT, 6], F32, tag="bnst")
            nc.vector.bn_stats(out=st[:], in_=xsrc[:, ti, :])
            nc.vector.bn_aggr(out=mvs[:, ti, :], in_=st[:])

    def ln_coeffs(mvs, rstd, nmr):
        """rstd = 1/sqrt(var+eps), nmr = -mean*rstd (per tile)."""
        nc.scalar.activation(rstd[:], mvs[:, :, 1], AF.Ln, bias=epsv[0:PT, :])
        nc.scalar.activation(rstd[:], rstd[:], AF.Exp, scale=-0.5)
        nc.vector.scalar_tensor_tensor(out=nmr[:], in0=mvs[:, :, 0],
                                       scalar=-1.0, in1=rstd[:],
                                       op0=AL.mult, op1=AL.mult)

    def ln_apply_transpose(dump, rstd, nmr, lnx, tag):
        """Scalar ACT per tile (x*rstd - mean*rstd), PE transpose pairs,
        vector evac into C-major lnx."""
        for tp2 in range(NTI // 2):
            ztp = psT.tile([C, 2, PT], BF16, tag="tp")
            for k in range(2):
                ti = 2 * tp2 + k
                z = small.tile([PT, C], BF16, tag=tag)
                nc.scalar.activation(z[:], dump[:, ti, :], AF.Identity,
                                     bias=nmr[:, ti:ti + 1],
                                     scale=rstd[:, ti:ti + 1])
                nc.tensor.transpose(ztp[:, k, :], z[:], identb[0:PT, 0:PT])
            nc.vector.tensor_copy(lnx[:, bass.ds(224 * tp2, 224)],
                                  ztp.rearrange("p a b -> p (a b)"))

    def phase_A(img):
        # x in token-major tiles (tile ti = tokens [112*ti, +112) of image)
        x_tm = pimg.tile([PT, NTI, C], F32, tag="x_img")
        nc.sync.dma_start(
            out=x_tm[:, :, :],
            in_=x_in[img * L:(img + 1) * L].rearrange("(n p) c -> p n c", p=PT))
        xb = pimg.tile([PT, NTI, C], BF16, tag="xb")

        # ---- LN1 stats + coeffs + apply/transpose ----
        mvs1 = small.tile([PT, NTI, 2], F32, tag="mvs1")
        ln_stats(x_tm, mvs1)
        rstd1 = small.tile([PT, NTI], F32, tag="rstd1")
        nmr1 = small.tile([PT, NTI], F32, tag="nmr1")
        ln_coeffs(mvs1, rstd1, nmr1)
        lnx_h = pim1.tile([C, L], BF16, tag="lnx_h")
        ln_apply_transpose(x_tm, rstd1, nmr1, lnx_h, "zt")
        # branch-0 view: visit lnx_h columns in w-major order
        lnw_v = lnx_h.rearrange("p (h w) -> p w h", h=RESO)

        # ---- qkv (col-packed: br0 rows 0:64 via strided rhs, br1 rows 64:128) ----
        qT = pim1.tile([C, L], BF16, tag="qT")
        kT = pim1.tile([C, L], BF16, tag="kT")
        vT = pim1.tile([C, L], BF16, tag="vT")
        for ck in range(NCK):
            sl = bass.ts(ck, CK)
            wv8 = bass.ds(8 * ck, 8)
            pqk = psU.tile([C, 2, 512], F32, tag="u")
            pv = psU.tile([C, 2, 512], F32, tag="u")
            for half in range(2):
                src = lnw_v[:, wv8, :] if half == 0 else lnx_h[:, sl]
                hs = bass.ds(64 * half, 64)
                nc.tensor.matmul(pqk[hs, 0, 0:CK], wqkv[:, bass.ds(64 * half, 64)],
                                 src, start=True, stop=True,
                                 tile_position=(0, 64 * half))
                nc.tensor.matmul(pqk[hs, 1, 0:CK], wqkv[:, bass.ds(C + 64 * half, 64)],
                                 src, start=True, stop=True,
                                 tile_position=(0, 64 * half))
                nc.tensor.matmul(pv[hs, 0, 0:CK], wqkv[:, bass.ds(2 * C + 64 * half, 64)],
                                 src, start=True, stop=True,
                                 tile_position=(0, 64 * half))
            nc.scalar.activation(qT[:, sl], pqk[:, 0, 0:CK], AF.Identity, bias=s2q)
            nc.vector.tensor_copy(kT[:, sl], pqk[:, 1, 0:CK])
            nc.scalar.activation(vT[:, sl], pv[:, 0, 0:CK], AF.Identity, bias=s2v)

        # ---- v_tm: token-major v via PE transposes of vT windows ----
        v_tm = pim1.tile([PT, NWIN, C], BF16, tag="v_tm")
        for s4 in range(NWIN // 4):
            pvt = psU.tile([PT, 4, C], BF16, tag="u")
            for g in range(4):
                w = 4 * s4 + g
                nc.tensor.transpose(pvt[:, g, :], vT[:, bass.ts(w, WT)],
                                    identb[:, :])
            nc.vector.tensor_copy(v_tm[:, bass.ds(4 * s4, 4), :], pvt[:])

        # ---- x-shifted copies of vT for the LePE dx taps ----
        vs1 = pim1.tile([C, L], BF16, tag="vs1")
        vsm1 = pim1.tile([C, L], BF16, tag="vsm1")
        nc.sync.dma_start(out=vs1[:, 0:L - 1], in_=vT[:, 1:L])
        nc.sync.dma_start(out=vsm1[:, 1:L], in_=vT[:, 0:L - 1])
        vs1_v = vs1.rearrange("p (s y x) -> p s y x", s=NWIN, y=2)
        vsm1_v = vsm1.rearrange("p (s y x) -> p s y x", s=NWIN, y=2)
        nc.vector.memset(vs1_v[:, :, :, 55:56], 0.0)
        nc.vector.memset(vsm1_v[:, :, :, 0:1], 0.0)

        # ---- attention: per superchunk (4 windows), heads in pairs ----
        tmp_att = pim1.tile([C, L], BF16, tag="tmp_att")
        for s4 in range(NCK):
            pT_t = small.tile([WT, 4, 4 * WT], BF16, tag="pT")
            for hp in range(2):            # head pairs {0,1}, {2,3}
                sp = psU.tile([WT, 2, 512], F32, tag="u")
                for g in range(4):
                    wsl = bass.ts(4 * s4 + g, WT)
                    for hh in range(2):
                        h = 2 * hp + hh
                        hsl = bass.ds(32 * h, 32)
                        nc.tensor.matmul(sp[:, hh, bass.ds(112 * g, WT)],
                                         kT[hsl, wsl], qT[hsl, wsl],
                                         start=True, stop=True,
                                         tile_position=(32 * h, 0))
                nc.scalar.activation(pT_t[:, bass.ds(2 * hp, 2), :],
                                     sp[:, :, 0:4 * WT], AF.Exp)
            sa = psU.tile([C, 2, 512], F32, tag="u")
            sums = sa[:, 0, 0:CK]
            avp = sa[:, 1, 0:CK]
            for h in range(4):
                po = bass.ds(32 * h, 32)
                nc.tensor.matmul(sa[po, 0, 0:CK], onesb[:, 0:32], pT_t[:, h, :],
                                 start=True, stop=True, tile_position=(0, 32 * h))
                vsl = bass.ds(64 * (h // 2) + 32 * (h % 2), 32)
                for g in range(4):
                    nc.tensor.matmul(sa[po, 1, bass.ds(112 * g, WT)],
                                     v_tm[:, 4 * s4 + g, vsl],
                                     pT_t[:, h, bass.ts(g, WT)],
                                     start=True, stop=True,
                                     tile_position=(0, 32 * h))
            rec = small.tile([C, CK], F32, tag="rec")
            nc.vector.reciprocal_approx_fast(out=rec[:], in_=sums)
            nc.vector.tensor_tensor(out=tmp_att[:, bass.ts(s4, CK)], in0=avp,
                                    in1=rec[:], op=AL.mult)

        # ---- LePE: 8 aligned taps onto tmp_att, centre tap writes att_h ----
        # stripe views (c, s, y, x); tap index (dy+1)*3+(dx+1)
        va = vT.rearrange("p (s y x) -> p s y x", s=NWIN, y=2)
        aa = tmp_att.rearrange("p (s y x) -> p s y x", s=NWIN, y=2)

        def tap(dy, src_v):
            # src_v pre-shifted in x; dy selects cross-plane row
            if dy == 0:
                return aa[:, :, :, :], src_v[:, :, :, :]
            if dy == 1:
                return aa[:, :, 0:1, :], src_v[:, :, 1:2, :]
            return aa[:, :, 1:2, :], src_v[:, :, 0:1, :]

        for dy in (1, -1):
            for dx, sv in ((0, va), (1, vs1_v), (-1, vsm1_v)):
                o, i = tap(dy, sv)
                nc.vector.scalar_tensor_tensor(
                    out=o, in0=i, scalar=taps[(dy + 1) * 3 + (dx + 1)], in1=o,
                    op0=AL.mult, op1=AL.add)
        for dx, sv in ((1, vs1_v), (-1, vsm1_v)):
            o, i = tap(0, sv)
            nc.vector.scalar_tensor_tensor(
                out=o, in0=i, scalar=taps[3 + (dx + 1)], in1=o,
                op0=AL.mult, op1=AL.add)
        # centre tap last: br1 contiguous, br0 scatters w-major -> h-major
        att_h = pimg.tile([C, L], BF16, tag="att_h")
        ah0 = att_h[0:64].rearrange("p (x s y) -> p s y x", x=RESO, s=NWIN)
        nc.vector.scalar_tensor_tensor(
            out=ah0, in0=va[0:64], scalar=taps[4][0:64], in1=aa[0:64],
            op0=AL.mult, op1=AL.add)
        nc.vector.scalar_tensor_tensor(
            out=att_h[64:128, :], in0=vT[64:128, :], scalar=taps[4][64:128],
            in1=tmp_att[64:128, :], op0=AL.mult, op1=AL.add)

        if "attT" in dbg_outs and img == 0:
            dc = small.tile([C, L], F32, tag="dbg")
            nc.vector.tensor_copy(dc[:], att_h[:])
            nc.sync.dma_start(out=dbg_outs["attT"], in_=dc[:])
        return x_tm, xb, att_h

    def phase_B(img, state):
        x_tm, xb, att_h = state
        # ---- proj ----
        projT = pim1.tile([C, L], BF16, tag="projT")
        for ck in range(NCK):
            sl = bass.ts(ck, CK)
            pp = psU.tile([C, 2, 512], F32, tag="u")
            nc.tensor.matmul(pp[:, 0, 0:CK], wproj[:], att_h[:, sl],
                             start=True, stop=True)
            nc.scalar.activation(projT[:, sl], pp[:, 0, 0:CK], AF.Identity,
                                 bias=projb)

        # ---- residual 1 (x + proj, bf16 out, batched pairs) ----
        for tp2 in range(NTI // 2):
            ptp = psT.tile([PT, 2, C], BF16, tag="tp")
            for k in range(2):
                ti = 2 * tp2 + k
                nc.tensor.transpose(ptp[:, k, :], projT[:, bass.ts(ti, PT)],
                                    identb[:, 0:C])
            xsl = bass.ds(2 * tp2, 2)
            nc.vector.tensor_tensor(out=xb[:, xsl, :], in0=ptp[:],
                                    in1=x_tm[:, xsl, :], op=AL.add)

        # ---- LN2 stats + coeffs + apply/transpose ----
        mvs2 = small.tile([PT, NTI, 2], F32, tag="mvs2")
        ln_stats(xb, mvs2)
        rstd2 = small.tile([PT, NTI], F32, tag="rstd2")
        nmr2 = small.tile([PT, NTI], F32, tag="nmr2")
        ln_coeffs(mvs2, rstd2, nmr2)
        lnx2 = pim1.tile([C, L], BF16, tag="lnx2")
        ln_apply_transpose(xb, rstd2, nmr2, lnx2, "z2t")

        # ---- MLP + residual 2 ----
        for ck in range(NCK):
            sl = bass.ts(ck, CK)
            hb = small.tile([C, 4, CK], BF16, tag="hb")
            for hp in range(2):
                ph = psU.tile([C, 2, 512], F32, tag="u")
                for hh in range(2):
                    h = 2 * hp + hh
                    nc.tensor.matmul(ph[:, hh, 0:CK], wfc1[:, bass.ds(128 * h, 128)],
                                     lnx2[:, sl], start=True, stop=True)
                    nc.scalar.activation(hb[:, h, :], ph[:, hh, 0:CK], AF.Gelu,
                                         bias=fc1b[h])
            p2 = psU.tile([C, 2, 512], F32, tag="u")
            for h in range(4):
                nc.tensor.matmul(p2[:, 0, 0:CK], wfc2[:, h, :], hb[:, h, :],
                                 start=(h == 0), stop=(h == 3))
            f2 = small.tile([C, CK], BF16, tag="f2")
            nc.vector.tensor_scalar(out=f2[:], in0=p2[:, 0, 0:CK], scalar1=fc2b,
                                    scalar2=None, op0=AL.add)
            for tj2 in range(2):
                ftp = psT.tile([PT, 2, C], BF16, tag="tp")
                for k in range(2):
                    tj = 2 * tj2 + k
                    nc.tensor.transpose(ftp[:, k, :], f2[:, bass.ts(tj, PT)],
                                        identb[:, 0:C])
                ti = 4 * ck + 2 * tj2
                nc.vector.tensor_tensor(out=x_tm[:, bass.ds(ti, 2), :],
                                        in0=ftp[:], in1=xb[:, bass.ds(ti, 2), :],
                                        op=AL.add)

        nc.sync.dma_start(
            out=out_t[img * L:(img + 1) * L].rearrange("(n p) c -> p n c", p=PT),
            in_=x_tm[:, :, :])

    # Skewed software pipeline: emit A(i) then B(i-1) so each engine's
    # stream interleaves independent work from adjacent images.
    state = [None] * IMG
    for i in range(IMG + 1):
        if i < IMG:
            state[i] = phase_A(i)
        if i >= 1:
            phase_B(i - 1, state[i - 1])


def _prep_inputs(inputs):
    """Host-side weight preprocessing (fp64 for exact folds)."""
    g1 = inputs["norm1_g"].astype(np.float64)
    b1 = inputs["norm1_b"].astype(np.float64)
    g2 = inputs["norm2_g"].astype(np.float64)
    b2 = inputs["norm2_b"].astype(np.float64)
    qkv_w = inputs["qkv_w"].astype(np.float64)
    proj_w = inputs["proj_w"].astype(np.float64)
    fc1_w = inputs["fc1_w"].astype(np.float64)
    fc2_w = inputs["fc2_w"].astype(np.float64)
    scale = HD ** -0.5

    wqkv = g1[:, None] * qkv_w
    s2 = b1 @ qkv_w
    wqkv[:, 0:C] *= scale
    s2q = s2[0:C] * scale
    s2k = s2[C:2 * C]
    s2v = s2[2 * C:3 * C]

    # LePE taps in stripe coords (y = stripe row in {0,1}, x = along stripe):
    # br1 (rows 64:128, h-major): (y,x) = (img_y, img_x) -> w1[dy+1, dx+1]
    # br0 (rows 0:64, w-major):  (y,x) = (img_x, img_y) -> transposed kernel
    w0 = inputs["conv_w0"].astype(np.float64)[:, 0]
    w1 = inputs["conv_w1"].astype(np.float64)[:, 0]
    taps = np.zeros((C, 9))
    for dy in (-1, 0, 1):
        for dx in (-1, 0, 1):
            ti = (dy + 1) * 3 + (dx + 1)
            taps[0:64, ti] = w0[:, dx + 1, dy + 1]
            taps[64:128, ti] = w1[:, dy + 1, dx + 1]

    cb = np.concatenate([inputs["conv_b0"], inputs["conv_b1"]]).astype(np.float64)
    projb_eff = inputs["proj_b"].astype(np.float64) + (s2v + cb) @ proj_w

    wfc1 = g2[:, None] * fc1_w
    fc1b_eff = b2 @ fc1_w + inputs["fc1_b"].astype(np.float64)

    vecs = np.zeros((C, 19))
    vecs[:, 0], vecs[:, 1], vecs[:, 2] = s2q, s2k, s2v
    vecs[:, 3], vecs[:, 4] = projb_eff, inputs["fc2_b"].astype(np.float64)
    vecs[:, 5] = EPS
    vecs[:, 6:15] = taps
    for h in range(4):
        vecs[:, 15 + h] = fc1b_eff[128 * h:128 * (h + 1)]

    return {
        "wqkv": np.ascontiguousarray(wqkv, np.float32),
        "wproj": np.ascontiguousarray(proj_w, np.float32),
        "wfc1": np.ascontiguousarray(wfc1, np.float32),
        "wfc2": np.ascontiguousarray(fc2_w, np.float32),
        "vecs": np.ascontiguousarray(vecs, np.float32),
    }


_CACHE = {}


class _Bacc(bacc.Bacc):
    """Bacc with the combined Ln+Exp activation-table set preferred, so the
    attention's Exp and the LN-rstd Ln/Exp stay on one table."""

    def insert_act_table_loads(self):
        import concourse.mybir as _mb
        from concourse.hw_specs import get_activation_tables as _gat
        from concourse.bacc import _bass_rust as _br
        has_activation = any(
            isinstance(i, _mb.InstActivation)
            for b in self.main_func.blocks
            for i in b.instructions
        )
        if not has_activation:
            return
        tables = list(_gat(self.m.arch).items())
        out = []
        for name, fns in tables:
            if name == "natural_log_exp_and_others":
                out.append((name, fns))
                continue
            if name in ("exp_and_others", "natural_log"):
                fns = {f for f in fns
                       if getattr(f, "name", str(f)) not in ("Exp", "Ln")}
            out.append((name, fns))
        _br.insert_act_table_loads(self, out)


def _get_nc(dbg=()):
    key = tuple(dbg)
    if key not in _CACHE:
        nc = _Bacc()
        build(nc, dbg)
        nc.finalize()
        _CACHE[key] = nc
    return _CACHE[key]


def kernel(**inputs):
    nc = _get_nc(_DBG[0] if _DBG else ())
    w = _prep_inputs(inputs)
    x = np.asarray(inputs["x"], np.float32)
    in_maps = []
    for c in range(N_CORES):
        m = dict(w)
        m["x"] = np.ascontiguousarray(x[c * IMG:(c + 1) * IMG].reshape(T, C))
        in_maps.append(m)
    trace = os.environ.get("KER_TRACE", "0") == "1"
    r = run_bass_kernel_spmd(nc, in_maps, list(range(N_CORES)), trace=trace)
    out = np.concatenate([r.results[c]["out"].reshape(IMG, L, C)
                          for c in range(N_CORES)], axis=0)
    kernel.last_results = r
    return out


_DBG = []
